# revision 51
# baseline (speedup 1.0000x reference)
"""Causal GQA self-attention kernel for Trainium2 (8 NeuronCores).

Sharding: 8 cores = batch (2) x kv-head-group (4). Each core computes, for
its (batch b, kv group g): the Q projection for the group's 4 query heads,
K/V projections for its kv head, causal flash attention for those heads,
and the partial output projection (rows of Wo for its heads). The host
sums the 4 partial outputs per batch element.

Schedule (v7):
  - all inputs host-cast to bf16 (halves input DMA; bf16 matmuls run at
    the same 1 cycle/row as fp32r but DVE elementwise goes 2x); output
    DMA'd as bf16 and upcast on the host.
  - startup: DMA order wk -> chunk-0 x pieces -> wv -> wq so the first
    K-projection matmul issues ~3us in; phase-1 PSUM->SBUF copies are
    split across ACT and DVE so the attention phase is not blocked on a
    trailing ACT copy queue.
  - attention is q-chunk-outer / head-inner with a deep software
    pipeline: S matmuls run up to 4 PSUM banks ahead, Y matmuls are
    deferred la_look+2 strips behind their exp, and leftover Ys ride
    into the NEXT head's strip stream (cross-engine dependency latency
    on real HW is ~1-2us per hop, so everything consumed from another
    engine must be produced several strips earlier).
  - causal masking multiplies only the 128-wide diagonal triangle block
    (on Pool); diagonal strips narrow S/exp/Y widths (min 256 free for
    full-rate matmuls).
  - softmax denominator: single bf16 accumulator on DVE (2x bf16 mode);
    one ones-matmul per chunk replicates l to all partitions; the
    epilogue (rps/reciprocal/divide) is deferred into the next chunk's
    stream (flushed at strip 3).
  - output projection (contract head dim against resident Wo rows) for
    chunk c is interleaved behind chunk c+1's attention heads (only 2
    tiles during q3, where ACT is exp-saturated; the rest at the tail);
    PSUM->SBUF copies are spread across whichever engines are idle in
    that window; one merged DMA per 128-row tile.
"""
import math

import numpy as np
import ml_dtypes

import concourse.bass as bass
import concourse.mybir as mybir
from concourse import bacc
from concourse.tile import TileContext
from concourse.masks import make_identity
from concourse.bass_utils import run_bass_kernel_spmd

F32 = mybir.dt.float32
BF16 = mybir.dt.bfloat16

E = 2048          # embedding dim
T = 2048          # sequence length
D = 128           # head dim
G = 4             # query heads per core (= GQA group size)
C = G * D         # 512 projected columns per core
KT = E // 128     # 16 contraction strips
NTCH = T // 512   # 4 t-chunks
SCALE = 1.0 / math.sqrt(D)


def build_attn(repeat: int = 1, phases: int = 3, sps_bufs: int = 3,
               o_bufs: int = 2, p_bufs: int = 8, xblk_bufs: int = 2,
               y_bufs: int = 3, la_look: int = 3, ot_bufs: int = 3):
    nc = bacc.Bacc()
    xT = nc.dram_tensor("xT", [E, T], BF16, kind="ExternalInput")
    wq = nc.dram_tensor("wq", [E, C], BF16, kind="ExternalInput")
    wk = nc.dram_tensor("wk", [E, D], BF16, kind="ExternalInput")
    wv = nc.dram_tensor("wv", [E, D], BF16, kind="ExternalInput")
    wo = nc.dram_tensor("wo", [C, E], BF16, kind="ExternalInput")
    tri = nc.dram_tensor("tri", [128, 128], BF16, kind="ExternalInput")
    out = nc.dram_tensor("out", [T, E], BF16, kind="ExternalOutput")

    xT3 = xT.rearrange("(ko p) t -> p ko t", p=128)
    wq3 = wq.rearrange("(ko p) c -> p ko c", p=128)
    wk3 = wk.rearrange("(ko p) d -> p ko d", p=128)
    wv3 = wv.rearrange("(ko p) d -> p ko d", p=128)
    wo3 = wo.rearrange("(g p) e -> p g e", p=128)

    with TileContext(nc) as tc:
        with tc.tile_pool(name="persist", bufs=1) as persist:
            qT_sb = persist.tile([128, G, T], BF16)
            kT_sb = persist.tile([128, T], BF16)
            v_sb = persist.tile([128, 16, 128], BF16)
            yT_sb = persist.tile([128, G, T], BF16)
            tri_sb = persist.tile([128, 128], BF16)
            scratch = persist.tile([128, 128], F32)
            ones_b = persist.tile([128, 128], BF16)
            ident_b = persist.tile([128, 128], BF16)
            nc.vector.memset(scratch[:], 1.0)
            nc.scalar.copy(ones_b[:], scratch[:])
            make_identity(nc, scratch[:])
            nc.scalar.copy(ident_b[:], scratch[:])

            def body():
                if phases < 1:
                    nc.sync.dma_start(tri_sb[:], tri[:])
                    nc.sync.dma_start(out[0:128, 0:128], tri_sb[:])
                    return
                # ---------------- Phase 1: projections ----------------
                with tc.tile_pool(name="wqkv", bufs=1) as wpool, \
                     tc.tile_pool(name="xblk", bufs=xblk_bufs) as xpool, \
                     tc.tile_pool(name="vt", bufs=2) as vtpool, \
                     tc.tile_pool(name="ps1", bufs=6, space="PSUM") as ps1, \
                     tc.tile_pool(name="ps1t", bufs=2, space="PSUM") as ps1t:
                    wq_sb = wpool.tile([128, KT, C], BF16)
                    wk_sb = wpool.tile([128, KT, D], BF16)
                    wv_sb = wpool.tile([128, KT, D], BF16)
                    # startup DMA order: wk, chunk-0 x pieces, wv, wq
                    nc.sync.dma_start(wk_sb[:], wk3)
                    xblk0 = xpool.tile([128, KT, 512], BF16, tag="xb")
                    for kq in range(4):
                        nc.sync.dma_start(xblk0[:, 4 * kq:4 * kq + 4, :],
                                          xT3[:, 4 * kq:4 * kq + 4, 0:512])
                        if kq == 0:
                            nc.sync.dma_start(wv_sb[:], wv3)
                    for cq in range(G):
                        nc.sync.dma_start(wq_sb[:, :, cq * 128:(cq + 1) * 128],
                                          wq3[:, :, cq * 128:(cq + 1) * 128])
                    nc.sync.dma_start(tri_sb[:], tri[:])

                    for tch in range(NTCH):
                        t0 = tch * 512
                        if tch == 0:
                            xblk = xblk0
                        else:
                            xblk = xpool.tile([128, KT, 512], BF16, tag="xb")
                            for kq in range(4):
                                nc.sync.dma_start(
                                    xblk[:, 4 * kq:4 * kq + 4, :],
                                    xT3[:, 4 * kq:4 * kq + 4, t0:t0 + 512])
                        # kT chunk
                        psk = ps1.tile([128, 512], F32, tag="psp")
                        for k in range(KT):
                            nc.tensor.matmul(psk[:], (wk_sb[:, k, :]),
                                             (xblk[:, k, :]),
                                             start=(k == 0), stop=(k == KT - 1))
                        nc.vector.tensor_copy(kT_sb[:, t0:t0 + 512], psk[:])
                        # vT chunk -> v strips (natural [t, d]) via PE transpose
                        psv = ps1.tile([128, 512], F32, tag="psp")
                        for k in range(KT):
                            nc.tensor.matmul(psv[:], (wv_sb[:, k, :]),
                                             (xblk[:, k, :]),
                                             start=(k == 0), stop=(k == KT - 1))
                        vt_t = vtpool.tile([128, 512], BF16)
                        nc.scalar.copy(vt_t[:], psv[:])
                        pst = ps1t.tile([128, 512], BF16)
                        for i in range(4):
                            nc.tensor.transpose(pst[:, i * 128:(i + 1) * 128],
                                                vt_t[:, i * 128:(i + 1) * 128],
                                                ident_b[:])
                        nc.vector.tensor_copy(v_sb[:, tch * 4:(tch + 1) * 4, :],
                                              pst[:])
                        # qT chunks (4 head columns)
                        for c in range(G):
                            psq = ps1.tile([128, 512], F32, tag="psp")
                            for k in range(KT):
                                nc.tensor.matmul(
                                    psq[:], (wq_sb[:, k, c * 128:(c + 1) * 128]),
                                    (xblk[:, k, :]),
                                    start=(k == 0), stop=(k == KT - 1))
                            if c % 2 == 0:
                                nc.scalar.copy(qT_sb[:, c, t0:t0 + 512],
                                               psq[:])
                            else:
                                nc.vector.tensor_copy(
                                    qT_sb[:, c, t0:t0 + 512], psq[:])

                if phases < 2:
                    nc.sync.dma_start(out[0:128, 0:T], kT_sb[:])
                    nc.sync.dma_start(out[128:256, 0:T], qT_sb[:, 0, :])
                    return
                # ------------- Phase 2+3: attention + out-proj --------
                with tc.tile_pool(name="wo", bufs=1) as wopool, \
                     tc.tile_pool(name="sb23", bufs=1) as sb23, \
                     tc.tile_pool(name="ps23", bufs=1, space="PSUM") as ps23:
                    wo_sb = wopool.tile([128, G, E], BF16)
                    nc.sync.dma_start(wo_sb[:], wo3)

                    pending = [None]

                    def flush():
                        if pending[0] is not None:
                            pending[0]()
                            pending[0] = None

                    def emit_tile(tt, cp_engines, split_dma=False):
                        # one 128-row output tile: 4 e-chunks of matmuls,
                        # PSUM->SBUF copies spread over the engines that are
                        # idle in this window, one merged 1MB DMA (split per
                        # e-chunk for the last tile to shorten the tail)
                        ot = sb23.tile([128, E], BF16, tag="ot", bufs=ot_bufs)
                        for ech in range(4):
                            e0 = ech * 512
                            pso = ps23.tile([128, 512], F32, tag="o",
                                            bufs=o_bufs)
                            for hh in range(G):
                                nc.tensor.matmul(
                                    pso[:],
                                    (yT_sb[:, hh, tt * 128:(tt + 1) * 128]),
                                    (wo_sb[:, hh, e0:e0 + 512]),
                                    start=(hh == 0), stop=(hh == G - 1))
                            eng = cp_engines[ech]
                            if eng == "a":
                                nc.scalar.copy(ot[:, e0:e0 + 512], pso[:])
                            else:
                                nc.vector.tensor_copy(ot[:, e0:e0 + 512],
                                                      pso[:])
                            if split_dma:
                                nc.sync.dma_start(
                                    out[tt * 128:(tt + 1) * 128,
                                        e0:e0 + 512],
                                    ot[:, e0:e0 + 512])
                        if not split_dma:
                            nc.sync.dma_start(out[tt * 128:(tt + 1) * 128, :],
                                              ot[:])

                    ydq = []  # deferred Y matmuls, carried across heads

                    def emit_y(item):
                        s, v0, p, yps, n = item
                        nc.tensor.matmul(
                            yps[:, v0:], (v_sb[:, s, :]), (p[:, v0:512]),
                            start=(s == 0), stop=(s == n - 1))

                    for q in range(NTCH):
                        tq0 = q * 512
                        n = 4 * (q + 1)
                        flush_gi = min(3, n - 1)
                        # early full strips accumulate on Pool (lb), the
                        # rest on DVE (la): keeps each serial l-chain
                        # engine-local, and DVE (the q2/q3 pacer on HW)
                        # sheds ~40% of its adds to the otherwise-idle Pool
                        n_pool = (n - 4) // 2
                        for h in range(G):
                            la = sb23.tile([128, 512], BF16, tag="la", bufs=2)
                            if n_pool:
                                lb = sb23.tile([128, 512], BF16, tag="lb",
                                               bufs=2)
                            la_started = False
                            yps = ps23.tile([128, 512], F32, tag="y",
                                            bufs=y_bufs)
                            for s in range(n):
                                o = s - (n - 4)
                                v0 = max(0, 128 * o)
                                cs = 256 if o == 3 else v0
                                sps = ps23.tile([128, 512], F32, tag="s",
                                                bufs=sps_bufs)
                                nc.tensor.matmul(
                                    sps[:, 0:512 - cs],
                                    (kT_sb[:, s * 128:(s + 1) * 128]),
                                    (qT_sb[:, h, tq0 + cs:tq0 + 512]),
                                    start=True, stop=True)
                                if s == 0:
                                    # finish the previous head's deferred Ys
                                    while len(ydq) > la_look:
                                        emit_y(ydq.pop(0))
                                elif s == flush_gi:
                                    flush()
                                p = sb23.tile([128, 512], BF16, tag="p",
                                              bufs=p_bufs)
                                nc.scalar.activation(
                                    p[:, v0:512], sps[:, v0 - cs:512 - cs],
                                    mybir.ActivationFunctionType.Exp,
                                    scale=SCALE)
                                if o >= 0:
                                    nc.gpsimd.tensor_mul(
                                        p[:, v0:v0 + 128], p[:, v0:v0 + 128],
                                        tri_sb[:])
                                if s < n_pool:
                                    if s == 0:
                                        nc.gpsimd.tensor_copy(lb[:], p[:])
                                    else:
                                        nc.gpsimd.tensor_add(lb[:], lb[:],
                                                             p[:])
                                elif not la_started:
                                    nc.vector.tensor_copy(la[:, v0:],
                                                          p[:, v0:])
                                    la_started = True
                                else:
                                    nc.vector.tensor_add(
                                        la[:, v0:], la[:, v0:], p[:, v0:])
                                ydq.append((s, v0, p, yps, n))
                                while len(ydq) > la_look + 2:
                                    emit_y(ydq.pop(0))
                            # drain to lookahead depth; the rest ride into
                            # the next head's strip stream
                            while len(ydq) > la_look:
                                emit_y(ydq.pop(0))

                            def make_epi(h=h, tq0=tq0, la=la, yps=yps,
                                         lb=(lb if n_pool else None),
                                         drain=list(ydq)):
                                def epi():
                                    for item in drain:
                                        if item in ydq:
                                            ydq.remove(item)
                                            emit_y(item)
                                    rps = ps23.tile([128, 512], F32, tag="o",
                                                    bufs=o_bufs)
                                    nc.tensor.matmul(rps[:], (ones_b[:]),
                                                     (la[:]), start=True,
                                                     stop=(lb is None))
                                    if lb is not None:
                                        nc.tensor.matmul(rps[:], (ones_b[:]),
                                                         (lb[:]), start=False,
                                                         stop=True)
                                    rinv = sb23.tile([128, 512], F32,
                                                     tag="rinv", bufs=2)
                                    nc.vector.reciprocal(rinv[:], rps[:])
                                    nc.vector.tensor_mul(
                                        yT_sb[:, h, tq0:tq0 + 512], yps[:],
                                        rinv[:])
                                return epi

                            pending[0] = make_epi()
                            # interleave out-proj of chunk q-1; during q3
                            # only 2 tiles (ACT is near-saturated there),
                            # the rest go to the tail
                            if q == 1:
                                emit_tile(h, "aava")
                            elif q == 2:
                                emit_tile(4 + h, "vava")
                            elif q == 3 and h in (1, 3):
                                emit_tile(8 + h // 2, "avav")
                    for item in list(ydq):
                        emit_y(item)
                    ydq.clear()
                    flush()
                    if phases >= 3:
                        for tt in range(10, 16):
                            emit_tile(tt, "aava", split_dma=(tt == 15))

            if repeat == 1:
                body()
            else:
                for _rep in range(repeat):
                    if _rep:
                        tc.strict_bb_all_engine_barrier()
                    body()

    nc.compile()
    return nc


def _make_mask():
    r = np.arange(128)[:, None]
    c = np.arange(128)[None, :]
    return (c >= r).astype(ml_dtypes.bfloat16)


def make_in_maps(x, Wq, Wk, Wv, Wo):
    """Host-side shard + bf16 cast: one input map per core."""
    x = np.asarray(x, dtype=np.float32)
    B = x.shape[0]
    assert x.shape == (B, T, E)
    xTh = np.ascontiguousarray(np.transpose(x, (0, 2, 1))).astype(
        ml_dtypes.bfloat16)
    Wqb = np.asarray(Wq, np.float32).astype(ml_dtypes.bfloat16)
    Wkb = np.asarray(Wk, np.float32).astype(ml_dtypes.bfloat16)
    Wvb = np.asarray(Wv, np.float32).astype(ml_dtypes.bfloat16)
    Wob = np.asarray(Wo, np.float32).astype(ml_dtypes.bfloat16)
    mask_np = _make_mask()
    in_maps = []
    for core in range(8):
        b, g = divmod(core, 4)
        b = b % B
        in_maps.append({
            "xT": xTh[b],
            "wq": np.ascontiguousarray(Wqb[:, g * C:(g + 1) * C]),
            "wk": np.ascontiguousarray(Wkb[:, g * D:(g + 1) * D]),
            "wv": np.ascontiguousarray(Wvb[:, g * D:(g + 1) * D]),
            "wo": np.ascontiguousarray(Wob[g * C:(g + 1) * C, :]),
            "tri": mask_np,
        })
    return in_maps


_NC = None


def kernel(x, Wq, Wk, Wv, Wo):
    global _NC
    if _NC is None:
        _NC = build_attn(repeat=1)
    nc = _NC

    B = np.asarray(x).shape[0]
    in_maps = make_in_maps(x, Wq, Wk, Wv, Wo)
    res = run_bass_kernel_spmd(nc, in_maps, list(range(8))).results
    outp = np.empty((B, T, E), dtype=np.float32)
    for b in range(B):
        acc = res[4 * b]["out"].astype(np.float64)
        for g in range(1, 4):
            acc += res[4 * b + g]["out"]
        outp[b] = acc.astype(np.float32)
    return outp


# revision 52
# speedup vs baseline: 1.0780x; 1.0780x over previous
"""Causal GQA self-attention kernel for Trainium2 (8 NeuronCores).

Sharding: 8 cores = batch (2) x kv-head-group (4). Each core computes, for
its (batch b, kv group g): the Q projection for the group's 4 query heads,
K/V projections for its kv head, causal flash attention for those heads,
and the partial output projection (rows of Wo for its heads). The host
sums the 4 partial outputs per batch element.

Schedule (v7):
  - all inputs host-cast to bf16 (halves input DMA; bf16 matmuls run at
    the same 1 cycle/row as fp32r but DVE elementwise goes 2x); output
    DMA'd as bf16 and upcast on the host.
  - startup: DMA order wk -> chunk-0 x pieces -> wv -> wq so the first
    K-projection matmul issues ~3us in; phase-1 PSUM->SBUF copies are
    split across ACT and DVE so the attention phase is not blocked on a
    trailing ACT copy queue.
  - attention is q-chunk-outer / head-inner with a deep software
    pipeline: S matmuls run up to 4 PSUM banks ahead, Y matmuls are
    deferred la_look+2 strips behind their exp, and leftover Ys ride
    into the NEXT head's strip stream (cross-engine dependency latency
    on real HW is ~1-2us per hop, so everything consumed from another
    engine must be produced several strips earlier).
  - causal masking multiplies only the 128-wide diagonal triangle block
    (on Pool); diagonal strips narrow S/exp/Y widths (min 256 free for
    full-rate matmuls).
  - softmax denominator: single bf16 accumulator on DVE (2x bf16 mode);
    one ones-matmul per chunk replicates l to all partitions; the
    epilogue (rps/reciprocal/divide) is deferred into the next chunk's
    stream (flushed at strip 3).
  - output projection (contract head dim against resident Wo rows) for
    chunk c is interleaved behind chunk c+1's attention heads (only 2
    tiles during q3, where ACT is exp-saturated; the rest at the tail);
    PSUM->SBUF copies are spread across whichever engines are idle in
    that window; one merged DMA per 128-row tile.
"""
import math

import numpy as np
import ml_dtypes

import concourse.bass as bass
import concourse.mybir as mybir
from concourse import bacc
from concourse.tile import TileContext
from concourse.masks import make_identity
from concourse.bass_utils import run_bass_kernel_spmd

F32 = mybir.dt.float32
BF16 = mybir.dt.bfloat16

E = 2048          # embedding dim
T = 2048          # sequence length
D = 128           # head dim
G = 4             # query heads per core (= GQA group size)
C = G * D         # 512 projected columns per core
KT = E // 128     # 16 contraction strips
NTCH = T // 512   # 4 t-chunks
SCALE = 1.0 / math.sqrt(D)


def build_attn(repeat: int = 1, phases: int = 3, sps_bufs: int = 3,
               o_bufs: int = 2, p_bufs: int = 8, xblk_bufs: int = 2,
               y_bufs: int = 3, la_look: int = 3, ot_bufs: int = 3):
    nc = bacc.Bacc()
    xT = nc.dram_tensor("xT", [E, T], BF16, kind="ExternalInput")
    wq = nc.dram_tensor("wq", [E, C], BF16, kind="ExternalInput")
    wk = nc.dram_tensor("wk", [E, D], BF16, kind="ExternalInput")
    wv = nc.dram_tensor("wv", [E, D], BF16, kind="ExternalInput")
    wo = nc.dram_tensor("wo", [C, E], BF16, kind="ExternalInput")
    tri = nc.dram_tensor("tri", [128, 128], BF16, kind="ExternalInput")
    out = nc.dram_tensor("out", [T, E], BF16, kind="ExternalOutput")

    xT3 = xT.rearrange("(ko p) t -> p ko t", p=128)
    wq3 = wq.rearrange("(ko p) c -> p ko c", p=128)
    wk3 = wk.rearrange("(ko p) d -> p ko d", p=128)
    wv3 = wv.rearrange("(ko p) d -> p ko d", p=128)
    wo3 = wo.rearrange("(g p) e -> p g e", p=128)

    with TileContext(nc) as tc:
        with tc.tile_pool(name="persist", bufs=1) as persist:
            qT_sb = persist.tile([128, G, T], BF16)
            kT_sb = persist.tile([128, T], BF16)
            v_sb = persist.tile([128, 16, 128], BF16)
            yT_sb = persist.tile([128, G, T], BF16)
            tri_sb = persist.tile([128, 128], BF16)
            scratch = persist.tile([128, 128], F32)
            ones_b = persist.tile([128, 128], BF16)
            ident_b = persist.tile([128, 128], BF16)
            nc.vector.memset(scratch[:], 1.0)
            nc.scalar.copy(ones_b[:], scratch[:])
            make_identity(nc, scratch[:])
            nc.scalar.copy(ident_b[:], scratch[:])

            def body():
                if phases < 1:
                    nc.sync.dma_start(tri_sb[:], tri[:])
                    nc.sync.dma_start(out[0:128, 0:128], tri_sb[:])
                    return
                # ---------------- Phase 1: projections ----------------
                with tc.tile_pool(name="wqkv", bufs=1) as wpool, \
                     tc.tile_pool(name="xblk", bufs=xblk_bufs) as xpool, \
                     tc.tile_pool(name="vt", bufs=2) as vtpool, \
                     tc.tile_pool(name="ps1", bufs=6, space="PSUM") as ps1, \
                     tc.tile_pool(name="ps1t", bufs=2, space="PSUM") as ps1t:
                    wq_sb = wpool.tile([128, KT, C], BF16)
                    wk_sb = wpool.tile([128, KT, D], BF16)
                    wv_sb = wpool.tile([128, KT, D], BF16)
                    # startup DMA order: wk, chunk-0 x pieces, wv, wq
                    nc.sync.dma_start(wk_sb[:], wk3)
                    xblk0 = xpool.tile([128, KT, 512], BF16, tag="xb")
                    for kq in range(4):
                        nc.sync.dma_start(xblk0[:, 4 * kq:4 * kq + 4, :],
                                          xT3[:, 4 * kq:4 * kq + 4, 0:512])
                        if kq == 0:
                            nc.sync.dma_start(wv_sb[:], wv3)
                    for cq in range(G):
                        nc.sync.dma_start(wq_sb[:, :, cq * 128:(cq + 1) * 128],
                                          wq3[:, :, cq * 128:(cq + 1) * 128])
                    nc.sync.dma_start(tri_sb[:], tri[:])

                    for tch in range(NTCH):
                        t0 = tch * 512
                        if tch == 0:
                            xblk = xblk0
                        else:
                            xblk = xpool.tile([128, KT, 512], BF16, tag="xb")
                            for kq in range(4):
                                nc.sync.dma_start(
                                    xblk[:, 4 * kq:4 * kq + 4, :],
                                    xT3[:, 4 * kq:4 * kq + 4, t0:t0 + 512])
                        # kT chunk
                        psk = ps1.tile([128, 512], F32, tag="psp")
                        for k in range(KT):
                            nc.tensor.matmul(psk[:], (wk_sb[:, k, :]),
                                             (xblk[:, k, :]),
                                             start=(k == 0), stop=(k == KT - 1))
                        nc.vector.tensor_copy(kT_sb[:, t0:t0 + 512], psk[:])
                        # vT chunk -> v strips (natural [t, d]) via PE transpose
                        psv = ps1.tile([128, 512], F32, tag="psp")
                        for k in range(KT):
                            nc.tensor.matmul(psv[:], (wv_sb[:, k, :]),
                                             (xblk[:, k, :]),
                                             start=(k == 0), stop=(k == KT - 1))
                        vt_t = vtpool.tile([128, 512], BF16)
                        nc.scalar.copy(vt_t[:], psv[:])
                        pst = ps1t.tile([128, 512], BF16)
                        for i in range(4):
                            nc.tensor.transpose(pst[:, i * 128:(i + 1) * 128],
                                                vt_t[:, i * 128:(i + 1) * 128],
                                                ident_b[:])
                        nc.vector.tensor_copy(v_sb[:, tch * 4:(tch + 1) * 4, :],
                                              pst[:])
                        # qT chunks (4 head columns)
                        for c in range(G):
                            psq = ps1.tile([128, 512], F32, tag="psp")
                            for k in range(KT):
                                nc.tensor.matmul(
                                    psq[:], (wq_sb[:, k, c * 128:(c + 1) * 128]),
                                    (xblk[:, k, :]),
                                    start=(k == 0), stop=(k == KT - 1))
                            if c % 2 == 0:
                                nc.scalar.copy(qT_sb[:, c, t0:t0 + 512],
                                               psq[:])
                            else:
                                nc.vector.tensor_copy(
                                    qT_sb[:, c, t0:t0 + 512], psq[:])

                if phases < 2:
                    nc.sync.dma_start(out[0:128, 0:T], kT_sb[:])
                    nc.sync.dma_start(out[128:256, 0:T], qT_sb[:, 0, :])
                    return
                # ------------- Phase 2+3: attention + out-proj --------
                with tc.tile_pool(name="wo", bufs=1) as wopool, \
                     tc.tile_pool(name="sb23", bufs=1) as sb23, \
                     tc.tile_pool(name="ps23", bufs=1, space="PSUM") as ps23:
                    wo_sb = wopool.tile([128, G, E], BF16)
                    nc.sync.dma_start(wo_sb[:], wo3)

                    pending = [None]

                    def flush():
                        if pending[0] is not None:
                            pending[0]()
                            pending[0] = None

                    def emit_tile(tt, cp_engines, split_dma=False):
                        # one 128-row output tile: 4 e-chunks of matmuls,
                        # PSUM->SBUF copies spread over the engines that are
                        # idle in this window, one merged 1MB DMA (split per
                        # e-chunk for the last tile to shorten the tail)
                        ot = sb23.tile([128, E], BF16, tag="ot", bufs=ot_bufs)
                        for ech in range(4):
                            e0 = ech * 512
                            pso = ps23.tile([128, 512], F32, tag="o",
                                            bufs=o_bufs)
                            for hh in range(G):
                                nc.tensor.matmul(
                                    pso[:],
                                    (yT_sb[:, hh, tt * 128:(tt + 1) * 128]),
                                    (wo_sb[:, hh, e0:e0 + 512]),
                                    start=(hh == 0), stop=(hh == G - 1))
                            eng = cp_engines[ech]
                            if eng == "a":
                                nc.scalar.copy(ot[:, e0:e0 + 512], pso[:])
                            else:
                                nc.vector.tensor_copy(ot[:, e0:e0 + 512],
                                                      pso[:])
                            if split_dma:
                                nc.sync.dma_start(
                                    out[tt * 128:(tt + 1) * 128,
                                        e0:e0 + 512],
                                    ot[:, e0:e0 + 512])
                        if not split_dma:
                            nc.sync.dma_start(out[tt * 128:(tt + 1) * 128, :],
                                              ot[:])

                    ydq = []  # deferred Y matmuls, carried across heads

                    def emit_y(item):
                        s, v0, p, yps, n = item
                        nc.tensor.matmul(
                            yps[:, v0:], (v_sb[:, s, :]), (p[:, v0:512]),
                            start=(s == 0), stop=(s == n - 1))

                    for q in range(NTCH):
                        tq0 = q * 512
                        n = 4 * (q + 1)
                        flush_gi = min(3, n - 1)
                        for h in range(G):
                            la = sb23.tile([128, 512], BF16, tag="la", bufs=2)
                            yps = ps23.tile([128, 512], F32, tag="y",
                                            bufs=y_bufs)
                            for s in range(n):
                                o = s - (n - 4)
                                v0 = max(0, 128 * o)
                                cs = 256 if o == 3 else v0
                                sps = ps23.tile([128, 512], F32, tag="s",
                                                bufs=sps_bufs)
                                nc.tensor.matmul(
                                    sps[:, 0:512 - cs],
                                    (kT_sb[:, s * 128:(s + 1) * 128]),
                                    (qT_sb[:, h, tq0 + cs:tq0 + 512]),
                                    start=True, stop=True)
                                if s == 0:
                                    # finish the previous head's deferred Ys
                                    while len(ydq) > la_look:
                                        emit_y(ydq.pop(0))
                                elif s == flush_gi:
                                    flush()
                                p = sb23.tile([128, 512], BF16, tag="p",
                                              bufs=p_bufs)
                                nc.scalar.activation(
                                    p[:, v0:512], sps[:, v0 - cs:512 - cs],
                                    mybir.ActivationFunctionType.Exp,
                                    scale=SCALE)
                                if o >= 0:
                                    nc.gpsimd.tensor_mul(
                                        p[:, v0:v0 + 128], p[:, v0:v0 + 128],
                                        tri_sb[:])
                                if s == 0:
                                    nc.vector.tensor_copy(la[:], p[:])
                                else:
                                    nc.vector.tensor_add(
                                        la[:, v0:], la[:, v0:], p[:, v0:])
                                ydq.append((s, v0, p, yps, n))
                                while len(ydq) > la_look + 2:
                                    emit_y(ydq.pop(0))
                            # drain to lookahead depth; the rest ride into
                            # the next head's strip stream
                            while len(ydq) > la_look:
                                emit_y(ydq.pop(0))

                            def make_epi(h=h, tq0=tq0, la=la, yps=yps,
                                         drain=list(ydq)):
                                def epi():
                                    for item in drain:
                                        if item in ydq:
                                            ydq.remove(item)
                                            emit_y(item)
                                    rps = ps23.tile([128, 512], F32, tag="o",
                                                    bufs=o_bufs)
                                    nc.tensor.matmul(rps[:], (ones_b[:]),
                                                     (la[:]),
                                                     start=True, stop=True)
                                    rinv = sb23.tile([128, 512], F32,
                                                     tag="rinv", bufs=2)
                                    nc.vector.reciprocal(rinv[:], rps[:])
                                    nc.vector.tensor_mul(
                                        yT_sb[:, h, tq0:tq0 + 512], yps[:],
                                        rinv[:])
                                return epi

                            pending[0] = make_epi()
                            # interleave out-proj of chunk q-1; during q3
                            # only 2 tiles (ACT is near-saturated there),
                            # the rest go to the tail
                            if q == 1:
                                emit_tile(h, "aava")
                            elif q == 2:
                                emit_tile(4 + h, "vava")
                            elif q == 3 and h in (1, 3):
                                emit_tile(8 + h // 2, "avav")
                    for item in list(ydq):
                        emit_y(item)
                    ydq.clear()
                    flush()
                    if phases >= 3:
                        for tt in range(10, 16):
                            emit_tile(tt, "aava", split_dma=(tt == 15))

            if repeat == 1:
                body()
            else:
                for _rep in range(repeat):
                    if _rep:
                        tc.strict_bb_all_engine_barrier()
                    body()

    nc.compile()
    return nc


def _make_mask():
    r = np.arange(128)[:, None]
    c = np.arange(128)[None, :]
    return (c >= r).astype(ml_dtypes.bfloat16)


def make_in_maps(x, Wq, Wk, Wv, Wo):
    """Host-side shard + bf16 cast: one input map per core."""
    x = np.asarray(x, dtype=np.float32)
    B = x.shape[0]
    assert x.shape == (B, T, E)
    xTh = np.ascontiguousarray(np.transpose(x, (0, 2, 1))).astype(
        ml_dtypes.bfloat16)
    Wqb = np.asarray(Wq, np.float32).astype(ml_dtypes.bfloat16)
    Wkb = np.asarray(Wk, np.float32).astype(ml_dtypes.bfloat16)
    Wvb = np.asarray(Wv, np.float32).astype(ml_dtypes.bfloat16)
    Wob = np.asarray(Wo, np.float32).astype(ml_dtypes.bfloat16)
    mask_np = _make_mask()
    in_maps = []
    for core in range(8):
        b, g = divmod(core, 4)
        b = b % B
        in_maps.append({
            "xT": xTh[b],
            "wq": np.ascontiguousarray(Wqb[:, g * C:(g + 1) * C]),
            "wk": np.ascontiguousarray(Wkb[:, g * D:(g + 1) * D]),
            "wv": np.ascontiguousarray(Wvb[:, g * D:(g + 1) * D]),
            "wo": np.ascontiguousarray(Wob[g * C:(g + 1) * C, :]),
            "tri": mask_np,
        })
    return in_maps


_NC = None


def kernel(x, Wq, Wk, Wv, Wo):
    global _NC
    if _NC is None:
        _NC = build_attn(repeat=1)
    nc = _NC

    B = np.asarray(x).shape[0]
    in_maps = make_in_maps(x, Wq, Wk, Wv, Wo)
    res = run_bass_kernel_spmd(nc, in_maps, list(range(8))).results
    outp = np.empty((B, T, E), dtype=np.float32)
    for b in range(B):
        acc = res[4 * b]["out"].astype(np.float64)
        for g in range(1, 4):
            acc += res[4 * b + g]["out"]
        outp[b] = acc.astype(np.float32)
    return outp
